# revision 1
# baseline (speedup 1.0000x reference)
"""GCN (2-layer, PyG GCNConv-style) on 8 Trainium2 NeuronCores.

Strategy (per spec sharding hint):
  - Destination nodes sharded 8 ways (6272 = 49*128 dst slots / core, last
    core partially filled); edges partitioned by destination shard on host.
  - Small weights replicated; every core computes the full first-layer
    linear transform h' = dinv * (x @ W1) (cheap) so the per-edge gather
    is purely local — no halo traffic for layer 1.
  - Per-edge rows fetched with dma_gather (int16 indices, 256B elements);
    the node->row permutation rowid = (n%128)*392 + n//128 makes rows for
    both layers live in one shared index space, so layer 1 and layer 2 use
    identical index tables and selection data. int16 range forces a
    low(<32768)/high row split: each dst tile accumulates its low chunks
    and high chunks in two PSUM passes joined in the epilogue.
  - Aggregation: edges grouped into 128-edge chunks per 128-dst-node tile;
    gathered rows are segment-summed on the PE via one-hot selection-matrix
    matmuls (sel[e,d] = (dst_e == d)) accumulating in PSUM.
  - Between layers, the [6272,16] shard features are AllGathered (1.6 MB),
    then expanded locally into the padded gather layout.
  - log_softmax epilogue on-chip; output sharded by dst, host concatenates.
"""

import os
import sys

import numpy as np

for _p in ("/opt/trn_rl_repo", "/root/.axon_site/_ro/trn_rl_repo"):
    if os.path.isdir(_p) and _p not in sys.path:
        sys.path.insert(0, _p)

import ml_dtypes  # noqa: E402
import concourse.bass as bass  # noqa: E402
import concourse.mybir as mybir  # noqa: E402
import concourse.tile as tile  # noqa: E402
from concourse.bass_utils import run_bass_kernel_spmd  # noqa: E402
from concourse.masks import make_identity  # noqa: E402
from concourse import library_config  # noqa: E402
import concourse.bass_isa as bass_isa  # noqa: E402

# ---------------- static problem config (hardcoded per contract) -------------
N = 50000
E = 800000
F = 64          # F_IN == F_HID
FO = 16         # F_OUT
NCORES = 8
P = 128
NT = 49                   # dst tiles per core
SH = NT * P               # 6272 dst slots per core (last core: 6096 real)
NTH = 392                 # node tiles for h' (50176 = 392*128 padded nodes)
NPAD = NTH * P            # 50176
XH = NTH // 2             # 196 tiles per half
XCOLS = XH * P            # 25088
G = 64                    # chunks per dma_gather instruction (8192 edges)
EPR = 128                 # padded row length (elements, bf16) = 256 bytes
LOWROWS = 32768           # int16 index range split

BF16 = ml_dtypes.bfloat16

_CACHE = {}


def _rowid(n):
    return (n % P) * NTH + n // P


def _pack_idx16(vals):
    """Edge-position-ordered values [Ctot*128] -> int16 idx table [128, Ctot*8].

    dma_gather reads index k from (partition k%16, col k//16), replicated
    across the 8 q7 cores (partition stripes of 16).
    """
    k = np.arange(vals.size)
    tbl = np.zeros((16, vals.size // 16), dtype=np.int16)
    tbl[k % 16, k // 16] = vals.astype(np.int16)
    return np.tile(tbl, (8, 1))


def _preprocess(x, edge_index, W1, b1, W2, b2):
    """Host-side graph preprocessing + input staging for all 8 cores."""
    src = np.concatenate([np.asarray(edge_index[0]), np.arange(N, dtype=np.int64)])
    dst = np.concatenate([np.asarray(edge_index[1]), np.arange(N, dtype=np.int64)])

    deg = np.bincount(dst, minlength=N).astype(np.float64)
    dinv = np.where(deg > 0, 1.0 / np.sqrt(deg), 0.0).astype(np.float32)

    core = dst // SH
    rows = _rowid(src)
    lo = rows < LOWROWS

    per_core = []
    cnt_lo = np.zeros((NCORES, NT), dtype=np.int64)
    cnt_hi = np.zeros((NCORES, NT), dtype=np.int64)
    for c in range(NCORES):
        m = core == c
        r_c = rows[m]
        d_c = dst[m] - c * SH
        t_c = d_c // P
        lo_c = lo[m]
        for t in range(NT):
            cnt_lo[c, t] = np.count_nonzero((t_c == t) & lo_c)
            cnt_hi[c, t] = np.count_nonzero((t_c == t) & ~lo_c)
        per_core.append((r_c, d_c, t_c, lo_c))

    # uniform chunk counts across cores (SPMD: one program for all cores)
    C_lo = ((cnt_lo.max(axis=0) + P - 1) // P).astype(np.int64)
    C_hi = ((cnt_hi.max(axis=0) + P - 1) // P).astype(np.int64)
    off_lo = np.concatenate([[0], np.cumsum(C_lo)])
    Clo_tot = int(off_lo[-1])
    off_hi = Clo_tot + np.concatenate([[0], np.cumsum(C_hi)])
    Ctot = int(off_hi[-1])

    # edge-position-ordered value arrays; position k = chunk*128 + lane
    idxv = np.zeros((NCORES, Ctot * P), dtype=np.int64)
    selv = np.full((NCORES, P, Ctot), -1.0, dtype=np.float32)
    for c in range(NCORES):
        r_c, d_c, t_c, lo_c = per_core[c]
        for t in range(NT):
            for is_lo in (True, False):
                m = (t_c == t) & (lo_c == is_lo)
                rr = r_c[m]
                dd = (d_c[m] % P).astype(np.float32)
                base = int(off_lo[t]) if is_lo else int(off_hi[t])
                i = np.arange(rr.size)
                pos = (base + i // P) * P + (i % P)
                idxv[c, pos] = rr - (0 if is_lo else LOWROWS)
                selv[c, i % P, base + i // P] = dd
    idx16 = np.stack([_pack_idx16(idxv[c]) for c in range(NCORES)])

    # xpack [128, XCOLS] bf16: partition (a*64+f), col j  <->  x_pad[a*XCOLS+j, f]
    xpad = np.zeros((NPAD, F), dtype=np.float32)
    xpad[:N] = np.asarray(x, dtype=np.float32)
    xpack = (
        xpad.reshape(2, XCOLS, F).transpose(0, 2, 1).reshape(P, XCOLS).astype(BF16)
    )

    dinv_h = np.zeros((P, NTH), dtype=np.float32)
    nn = np.arange(N)
    dinv_h[nn % P, nn // P] = dinv
    dinv_s = np.zeros((NCORES, P, NT), dtype=np.float32)
    for c in range(NCORES):
        ll = np.arange(min(SH, N - c * SH))
        dinv_s[c, ll % P, ll // P] = dinv[c * SH : c * SH + ll.size]

    common = {
        "xpack": xpack,
        "w1": np.concatenate([np.asarray(W1, np.float32)] * 2, axis=0).astype(BF16),
        "w2": np.asarray(W2, dtype=np.float32).astype(BF16),
        "b1r": np.broadcast_to(np.asarray(b1, np.float32), (P, F)).copy(),
        "b2r": np.broadcast_to(np.asarray(b2, np.float32), (P, FO)).copy(),
        "dinvh": dinv_h,
    }
    in_maps = []
    for c in range(NCORES):
        m = dict(common)
        m["dinvs"] = dinv_s[c]
        m["idx16"] = idx16[c]
        m["selv"] = selv[c]
        in_maps.append(m)

    cot_lo = [list(range(int(off_lo[t]), int(off_lo[t + 1]))) for t in range(NT)]
    cot_hi = [list(range(int(off_hi[t]), int(off_hi[t + 1]))) for t in range(NT)]
    return in_maps, Ctot, Clo_tot, cot_lo, cot_hi


_WAIT_LIMIT = int(os.environ.get("GCN_WAIT_LIMIT", "1"))


def _legalize_waits(nc, limit=None):
    """Split >limit semaphore waits into standalone NOPs on the same engine.

    Walrus codegen rejects instructions whose sync_info carries more wait
    conditions than the ISA sync fields hold ("Too many sync wait commands").
    A chain of no-ops each carrying <=limit waits is semantically identical
    (waits are AND conditions and the engine queue is in-order).
    """
    if limit is None:
        limit = _WAIT_LIMIT
    import bass_rust as _br

    uid = 0
    for fn in nc.m.functions:
        for bb in fn.blocks:
            out = []
            changed = False
            for ins in bb.instructions:
                si = ins.sync_info
                if si is not None and len(si.on_wait) > limit:
                    waits = list(si.on_wait)
                    excess, keep = waits[:-limit], waits[-limit:]
                    for i in range(0, len(excess), limit):
                        nop = mybir.InstNoOp(name=f"waitsplit_{uid}", ins=[], outs=[])
                        uid += 1
                        nop.engine = ins.engine
                        nop.sync_info = _br.SyncInfo(
                            on_wait=excess[i : i + limit], on_update=[]
                        )
                        out.append(nop)
                    ins.sync_info = _br.SyncInfo(
                        on_wait=keep, on_update=list(si.on_update)
                    )
                    changed = True
                out.append(ins)
            if changed:
                bb.instructions = out


def _build(Ctot, Clo_tot, cot_lo, cot_hi):
    dt = mybir.dt
    Alu = mybir.AluOpType
    Act = mybir.ActivationFunctionType

    nc = bass.Bass(num_devices=NCORES)

    # ---- I/O ----
    xpack_e = nc.dram_tensor("xpack", [P, XCOLS], dt.bfloat16, kind="ExternalInput")
    w1_e = nc.dram_tensor("w1", [2 * F, F], dt.bfloat16, kind="ExternalInput")
    w2_e = nc.dram_tensor("w2", [F, FO], dt.bfloat16, kind="ExternalInput")
    b1_e = nc.dram_tensor("b1r", [P, F], dt.float32, kind="ExternalInput")
    b2_e = nc.dram_tensor("b2r", [P, FO], dt.float32, kind="ExternalInput")
    dinvh_e = nc.dram_tensor("dinvh", [P, NTH], dt.float32, kind="ExternalInput")
    dinvs_e = nc.dram_tensor("dinvs", [P, NT], dt.float32, kind="ExternalInput")
    idx16_e = nc.dram_tensor("idx16", [P, Ctot * 8], dt.int16, kind="ExternalInput")
    selv_e = nc.dram_tensor("selv", [P, Ctot], dt.float32, kind="ExternalInput")
    out_e = nc.dram_tensor("out", [P, NT * FO], dt.float32, kind="ExternalOutput")
    debug = bool(int(os.environ.get("GCN_DEBUG", "0")))
    phases = int(os.environ.get("GCN_PHASES", "3"))
    if debug:
        dbg1_e = nc.dram_tensor(
            "dbg_out1", [P, NT * F], dt.bfloat16, kind="ExternalOutput"
        )
        dbgg_e = nc.dram_tensor(
            "dbg_g0", [P, G * EPR], dt.bfloat16, kind="ExternalOutput"
        )

    # ---- internal DRAM ----
    # padded-row layouts: node n -> row (n%128)*NTH + n//128, 256B rows,
    # only the leading F (layer 1) / FO (layer 2) columns are meaningful.
    h_dram = nc.dram_tensor("h_dram", [NPAD, EPR], dt.bfloat16)
    h2_dram = nc.dram_tensor("h2_dram", [NPAD, EPR], dt.bfloat16)
    cc_in = nc.dram_tensor("cc_in", [P, NT * FO], dt.bfloat16)
    cc_out = nc.dram_tensor(
        "cc_out", [NCORES, P, NT * FO], dt.bfloat16, addr_space="Shared"
    )

    NGR_LO = (Clo_tot + G - 1) // G
    NGR_HI = (Ctot - Clo_tot + G - 1) // G

    def chunk_rhs(gbufs, j, base_chunk, fw):
        g, jl = (j - base_chunk) // G, (j - base_chunk) % G
        return gbufs[g][:, jl * EPR : jl * EPR + fw]

    with tile.TileContext(nc) as tc:
        with tc.tile_pool(name="const", bufs=1) as cp:
            w1 = cp.tile([2 * F, F], dt.bfloat16, tag="w1")
            nc.sync.dma_start(out=w1[:], in_=w1_e[:, :])
            w2 = cp.tile([F, FO], dt.bfloat16, tag="w2")
            nc.sync.dma_start(out=w2[:], in_=w2_e[:, :])
            b1r = cp.tile([P, F], dt.float32, tag="b1r")
            nc.sync.dma_start(out=b1r[:], in_=b1_e[:, :])
            b2r = cp.tile([P, FO], dt.float32, tag="b2r")
            nc.sync.dma_start(out=b2r[:], in_=b2_e[:, :])
            dinvh = cp.tile([P, NTH], dt.float32, tag="dinvh")
            nc.sync.dma_start(out=dinvh[:], in_=dinvh_e[:, :])
            dinvs = cp.tile([P, NT], dt.float32, tag="dinvs")
            nc.sync.dma_start(out=dinvs[:], in_=dinvs_e[:, :])
            idx16 = cp.tile([P, Ctot * 8], dt.int16, tag="idx16")
            nc.sync.dma_start(out=idx16[:], in_=idx16_e[:, :])
            selv = cp.tile([P, Ctot], dt.float32, tag="selv")
            nc.sync.dma_start(out=selv[:], in_=selv_e[:, :])
            iota = cp.tile([P, P], dt.bfloat16, tag="iota")
            nc.gpsimd.iota(
                iota[:],
                pattern=[[1, P]],
                base=0,
                channel_multiplier=0,
                allow_small_or_imprecise_dtypes=True,
            )
            ident = cp.tile([P, P], dt.bfloat16, tag="ident")
            make_identity(nc, ident[:])
            out1 = cp.tile([P, NT * F], dt.bfloat16, tag="out1")
            acc1 = cp.tile([P, NT * F], dt.float32, tag="acc1")
            h2st = cp.tile([P, NT * FO], dt.bfloat16, tag="h2st")
            outst = cp.tile([P, NT * FO], dt.float32, tag="outst")

            tc.strict_bb_all_engine_barrier()
            # dma_gather lives in the Q7 "mlp" extended-instruction library.
            # bass's pseudo reload ships with an empty instr payload, which
            # walrus rejects ("ISA wrong length") — fill the 64B struct.
            _li = nc.gpsimd.load_library(library_config.mlp)
            _instr, _fx = bass_isa.isa_struct(
                nc.isa,
                nc.isa.Opcode.NEURON_ISA_TPB_OPCODE_PSEUDO_INST,
                {"pseudo_opcode": 2, "lib_index": library_config.mlp.index},
                struct_name="NEURON_ISA_TPB_PSEUDO_LIBRARY_RELOAD_INDEX_STRUCT",
            )
            _li.ins.instr = _instr

            # ---------------- Phase A: h' = bf16(dinv * (x @ W1)) ------------
            XB = 1792  # 14 node tiles per (block, half); 14 blocks
            with (
                tc.tile_pool(name="xa", bufs=3) as xpool,
                tc.tile_pool(name="ha", bufs=3) as hpool,
                tc.tile_pool(name="pha", bufs=4, space="PSUM") as phpool,
            ):
                for blk in range(0, XCOLS, XB):
                    xb = xpool.tile([P, XB], dt.bfloat16, tag="xb")
                    nc.sync.dma_start(out=xb[:], in_=xpack_e[:, blk : blk + XB])
                    for a in (0, 1):
                        nt_blk = XB // P
                        hst = hpool.tile([P, nt_blk * F], dt.bfloat16, tag="hst")
                        for m in range(nt_blk):
                            tg = a * XH + blk // P + m
                            ph = phpool.tile([P, F], dt.float32, tag="ph")
                            nc.tensor.matmul(
                                out=ph[:],
                                lhsT=xb[a * F : (a + 1) * F, m * P : (m + 1) * P],
                                rhs=w1[a * F : (a + 1) * F, :],
                                start=True,
                                stop=True,
                            )
                            nc.scalar.activation(
                                out=hst[:, m * F : (m + 1) * F],
                                in_=ph[:],
                                func=Act.Copy,
                                scale=dinvh[:, tg : tg + 1],
                            )
                        tg0 = a * XH + blk // P
                        # dest rows r = p*NTH + tg, tg in [tg0, tg0+nt_blk),
                        # first F cols; src order (p, m, f)
                        dst = bass.AP(
                            h_dram,
                            tg0 * EPR,
                            [[NTH * EPR, P], [EPR, nt_blk], [1, F]],
                        )
                        nc.sync.dma_start(out=dst, in_=hst[:])


            # ---------------- aggregation machinery --------------------------
            def aggregation(src_dram, fw, epilogue, dump_g0=False):
                """Two-pass (low rows / high rows) chunked aggregation.

                Low pass: psum -> acc1 (f32).  High pass: psum; epilogue(t, pt)
                consumes pt (high psum) + acc1 slice (low partial).
                """
                src_lo = src_dram.ap()  # [NPAD, EPR]
                src_hi = bass.AP(
                    src_dram,
                    LOWROWS * EPR,
                    [[EPR, NPAD - LOWROWS], [1, EPR]],
                )
                with (
                    tc.tile_pool(name="gb", bufs=4) as gpool,
                    tc.tile_pool(name="selp", bufs=10) as selpool,
                    tc.tile_pool(name="paggp", bufs=8, space="PSUM") as paggpool,
                ):
                    gbufs_lo = [None] * NGR_LO
                    gbufs_hi = [None] * NGR_HI

                    def issue(g, base_chunk, nchunks, srcv, store):
                        c0 = base_chunk + g * G
                        w = min(nchunks, (g + 1) * G) - g * G
                        gb = gpool.tile([P, G * EPR], dt.bfloat16, tag="gb")
                        nc.gpsimd.dma_gather(
                            out_ap=gb[:, : w * EPR].rearrange(
                                "p (s e) -> p s e", e=EPR
                            ),
                            in_ap=srcv,
                            idxs_ap=idx16[:, c0 * 8 : (c0 + w) * 8],
                            num_idxs=w * P,
                            num_idxs_reg=w * P,
                            elem_size=EPR,
                            single_packet=False,
                        )
                        store[g] = gb
                        if dump_g0 and base_chunk == 0 and g == 0:
                            nc.sync.dma_start(out=dbgg_e[:, :], in_=gb[:])

                    def build_sel(j):
                        sel = selpool.tile([P, P], dt.bfloat16, tag="sel")
                        nc.vector.tensor_scalar(
                            out=sel[:],
                            in0=iota[:],
                            scalar1=selv[:, j : j + 1],
                            scalar2=None,
                            op0=Alu.is_equal,
                        )
                        return sel

                    # ---- low pass: accumulate into acc1 ----
                    issued = 0
                    for t in range(NT):
                        if not cot_lo[t]:
                            nc.vector.memset(acc1[:, t * F : t * F + fw], 0.0)
                            continue
                        while issued * G < cot_lo[t][-1] + 1:
                            issue(issued, 0, Clo_tot, src_lo, gbufs_lo)
                            issued += 1
                        pt = paggpool.tile([P, F], dt.float32, tag="pagg")
                        for k, j in enumerate(cot_lo[t]):
                            sel = build_sel(j)
                            nc.tensor.matmul(
                                out=pt[:, :fw],
                                lhsT=sel[:],
                                rhs=chunk_rhs(gbufs_lo, j, 0, fw),
                                start=(k == 0),
                                stop=(k == len(cot_lo[t]) - 1),
                            )
                        nc.vector.tensor_copy(
                            out=acc1[:, t * F : t * F + fw], in_=pt[:, :fw]
                        )
                    # ---- high pass: psum + epilogue ----
                    issued = 0
                    for t in range(NT):
                        pt = paggpool.tile([P, F], dt.float32, tag="pagg")
                        if cot_hi[t]:
                            while issued * G < cot_hi[t][-1] - Clo_tot + 1:
                                issue(issued, Clo_tot, Ctot - Clo_tot, src_hi,
                                      gbufs_hi)
                                issued += 1
                            for k, j in enumerate(cot_hi[t]):
                                sel = build_sel(j)
                                nc.tensor.matmul(
                                    out=pt[:, :fw],
                                    lhsT=sel[:],
                                    rhs=chunk_rhs(gbufs_hi, j, Clo_tot, fw),
                                    start=(k == 0),
                                    stop=(k == len(cot_hi[t]) - 1),
                                )
                        else:
                            nc.vector.memset(pt[:, :fw], 0.0)
                        epilogue(t, pt)

            # ---------------- Phase B: layer-1 aggregation -------------------
            if phases >= 1:
              with tc.tile_pool(name="ep1", bufs=4) as ep1pool:

                def epi1(t, pt):
                    tmp = ep1pool.tile([P, F], dt.float32, tag="tmp")
                    nc.vector.tensor_tensor(
                        out=tmp[:],
                        in0=pt[:, :F],
                        in1=acc1[:, t * F : (t + 1) * F],
                        op=Alu.add,
                    )
                    nc.vector.tensor_scalar(
                        out=tmp[:],
                        in0=tmp[:],
                        scalar1=dinvs[:, t : t + 1],
                        scalar2=None,
                        op0=Alu.mult,
                    )
                    nc.vector.tensor_tensor(
                        out=tmp[:], in0=tmp[:], in1=b1r[:], op=Alu.add
                    )
                    nc.scalar.activation(
                        out=out1[:, t * F : (t + 1) * F], in_=tmp[:], func=Act.Relu
                    )

                aggregation(h_dram, F, epi1, dump_g0=debug)

            if debug:
                nc.sync.dma_start(out=dbg1_e[:, :], in_=out1[:])

            # ---------------- Phase C: h2' = bf16(dinv * (out1 @ W2)) --------
            if phases >= 2:
              with (
                tc.tile_pool(name="ptrp", bufs=2, space="PSUM") as ptrpool,
                tc.tile_pool(name="ph2p", bufs=2, space="PSUM") as ph2pool,
                tc.tile_pool(name="o1tp", bufs=2) as o1tpool,
            ):
                for t in range(NT):
                    ptr_ = ptrpool.tile([P, P], dt.bfloat16, tag="ptr")
                    nc.tensor.transpose(
                        out=ptr_[:F, :],
                        in_=out1[:, t * F : (t + 1) * F],
                        identity=ident[:],
                    )
                    o1T = o1tpool.tile([F, P], dt.bfloat16, tag="o1T")
                    nc.vector.tensor_copy(out=o1T[:], in_=ptr_[:F, :])
                    ph2 = ph2pool.tile([P, FO], dt.float32, tag="ph2")
                    nc.tensor.matmul(
                        out=ph2[:], lhsT=o1T[:], rhs=w2[:, :], start=True, stop=True
                    )
                    nc.scalar.activation(
                        out=h2st[:, t * FO : (t + 1) * FO],
                        in_=ph2[:],
                        func=Act.Copy,
                        scale=dinvs[:, t : t + 1],
                    )
                nc.sync.dma_start(out=cc_in[:, :], in_=h2st[:])

            if phases >= 2:
              nc.gpsimd.collective_compute(
                "AllGather",
                mybir.AluOpType.bypass,
                replica_groups=[list(range(NCORES))],
                ins=[cc_in.ap()],
                outs=[cc_out.ap()],
            )

            if phases >= 2:

              # expand cc_out [c, p, (t f)] -> h2_dram rows (p*NTH + c*NT + t),
            # first FO cols.  src order (c, p, t, f); dest AP same order.
              with tc.tile_pool(name="exp", bufs=1) as expool:
                xt = expool.tile([P, NCORES * NT * FO], dt.bfloat16, tag="xt")
                for c in range(NCORES):
                    nc.sync.dma_start(
                        out=xt[:, c * NT * FO : (c + 1) * NT * FO],
                        in_=cc_out[c],
                    )
                dst = bass.AP(
                    h2_dram,
                    0,
                    [[NTH * EPR, P], [NT * EPR, NCORES], [EPR, NT], [1, FO]],
                )
                nc.sync.dma_start(out=dst, in_=xt[:])


            # ---------------- Phase D: layer-2 aggregation + log_softmax -----
            if phases >= 3:
              with tc.tile_pool(name="ep2", bufs=4) as ep2pool:

                def epi2(t, pt):
                    tmp = ep2pool.tile([P, FO], dt.float32, tag="tmp2")
                    nc.vector.tensor_tensor(
                        out=tmp[:],
                        in0=pt[:, :FO],
                        in1=acc1[:, t * F : t * F + FO],
                        op=Alu.add,
                    )
                    nc.vector.tensor_scalar(
                        out=tmp[:],
                        in0=tmp[:],
                        scalar1=dinvs[:, t : t + 1],
                        scalar2=None,
                        op0=Alu.mult,
                    )
                    nc.vector.tensor_tensor(
                        out=tmp[:], in0=tmp[:], in1=b2r[:], op=Alu.add
                    )
                    mx = ep2pool.tile([P, 1], dt.float32, tag="mx")
                    nc.vector.reduce_max(
                        out=mx[:], in_=tmp[:], axis=mybir.AxisListType.X, negate=True
                    )
                    ex = ep2pool.tile([P, FO], dt.float32, tag="ex")
                    nc.scalar.activation(
                        out=ex[:], in_=tmp[:], func=Act.Exp, bias=mx[:, 0:1]
                    )
                    sm = ep2pool.tile([P, 1], dt.float32, tag="sm")
                    nc.vector.reduce_sum(
                        out=sm[:], in_=ex[:], axis=mybir.AxisListType.X
                    )
                    lg = ep2pool.tile([P, 1], dt.float32, tag="lg")
                    nc.scalar.activation(out=lg[:], in_=sm[:], func=Act.Ln)
                    nc.vector.tensor_scalar(
                        out=outst[:, t * FO : (t + 1) * FO],
                        in0=tmp[:],
                        scalar1=mx[:, 0:1],
                        scalar2=lg[:, 0:1],
                        op0=Alu.add,
                        op1=Alu.subtract,
                    )

                aggregation(h2_dram, FO, epi2)
            if phases >= 3:
                nc.sync.dma_start(out=out_e[:, :], in_=outst[:])
            else:
                nc.vector.memset(outst[:], 0.0)
                nc.sync.dma_start(out=out_e[:, :], in_=outst[:])

    _legalize_waits(nc)
    return nc


def kernel(x, edge_index, W1, b1, W2, b2, _trace=False, _trace_kwargs=None):
    in_maps, Ctot, Clo_tot, cot_lo, cot_hi = _preprocess(
        x, edge_index, W1, b1, W2, b2
    )
    key = (
        Ctot,
        Clo_tot,
        tuple(len(c) for c in cot_lo),
        tuple(len(c) for c in cot_hi),
    )
    if key not in _CACHE:
        _CACHE[key] = _build(Ctot, Clo_tot, cot_lo, cot_hi)
    nc = _CACHE[key]

    res = run_bass_kernel_spmd(
        nc,
        in_maps,
        core_ids=list(range(NCORES)),
        trace=_trace,
        **(_trace_kwargs or {}),
    )
    out = np.empty((N, FO), dtype=np.float32)
    for c in range(NCORES):
        o = np.asarray(res.results[c]["out"], dtype=np.float32)
        o = o.reshape(P, NT, FO).transpose(1, 0, 2).reshape(NT * P, FO)
        k = min(SH, N - c * SH)
        out[c * SH : c * SH + k] = o[:k]
    kernel._last_result = res
    return out



# revision 90
# speedup vs baseline: 1.3233x; 1.3233x over previous
"""GCN (2-layer, PyG GCNConv-style) on 8 Trainium2 NeuronCores.

Strategy:
  - Destination nodes sharded 8 ways (6272 dst slots / core); edges
    partitioned by destination shard on host.  Weights replicated; every
    core computes the full first-layer transform h' = (dinv*x) @ W1 (dinv
    pre-folded on host) so per-edge gathers are purely local.
  - Per-edge rows fetched with dma_gather (int16 indices, 256B rows);
    rowid = (n%128)*392 + n//128 shared by both layers.  int16 range
    forces a low(<32768)/high row split (two gather passes per layer).
  - Aggregation: all 49 dst-tile PSUM accumulators are open at once; the
    per-pass edge stream is lane-packed with static per-tile lane starts
    (max count across cores), so chunks need no per-tile rounding; chunks
    straddling a tile boundary issue one matmul per tile with a
    lane-masked selection matrix.  Selection matrices sel[e,d]=(dst_e==d)
    are built on DVE and used as one-hot matmul lhsT accumulating in PSUM.
  - Epilogue folds the dst-side dinv scale into a single Activation op
    (relu for layer 1); log_softmax epilogue for layer 2 on-chip.
  - Layer-2 transform h2' = dinv*(out1 @ W2) runs per-tile right after
    that tile's epilogue; the [6272,16] shard is AllGathered in 4 pieces
    overlapped with the tail of the layer-1 aggregation, then expanded
    into the padded row layout for the layer-2 gathers.
  - Output sharded by dst; host concatenates.
"""

import os
import sys

import numpy as np

for _p in ("/opt/trn_rl_repo", "/root/.axon_site/_ro/trn_rl_repo"):
    if os.path.isdir(_p) and _p not in sys.path:
        sys.path.insert(0, _p)

import ml_dtypes  # noqa: E402
import concourse.bass as bass  # noqa: E402
import concourse.mybir as mybir  # noqa: E402
import concourse.tile as tile  # noqa: E402
from concourse.tile_rust import add_dep_helper  # noqa: E402
from concourse.bass_utils import run_bass_kernel_spmd  # noqa: E402
from concourse.masks import make_identity  # noqa: E402
from concourse import library_config  # noqa: E402
import concourse.bass_isa as bass_isa  # noqa: E402

# ---------------- static problem config (hardcoded per contract) -------------
N = 50000
E = 800000
F = 64          # F_IN == F_HID
FO = 16         # F_OUT
NCORES = 8
P = 128
NT = 49                   # dst tiles per core
SH = NT * P               # 6272 dst slots per core (last core: 6096 real)
NTH = 392                 # node tiles (50176 = 392*128 padded nodes)
NPAD = NTH * P            # 50176
XH = NTH // 2             # 196 node tiles per half
XCOLS = XH * P            # 25088
G = 32                    # chunks per dma_gather instruction (4096 edges)
EPR = 128                 # padded row length (elements, bf16) = 256 bytes
LOWROWS = 32768           # int16 index range split
TPG = 7                   # node tiles per phase-A psum group
PB = [0, 10, 20, 31, 49]  # collective piece tile boundaries

BF16 = ml_dtypes.bfloat16

_CACHE = {}


def _rowid(n):
    # node n = c*SH + t*128 + p  ->  row (p*8+c)*49 + t; matches both the
    # phase-A h_dram write pattern and the padded cc_out AllGather layout
    # (rank innermost keeps self-loops balanced across the int16 lo/hi cut)
    c, r = n // SH, n % SH
    return ((r % P) * NCORES + c) * NT + r // P


def _pack_idx16(vals):
    """Edge-position-ordered values [C*128] -> int16 idx table [128, C*8].

    dma_gather reads index k from (partition k%16, col k//16), replicated
    across the 8 q7 cores (partition stripes of 16).
    """
    k = np.arange(vals.size)
    tbl = np.zeros((16, vals.size // 16), dtype=np.int16)
    tbl[k % 16, k // 16] = vals.astype(np.int16)
    return np.tile(tbl, (8, 1))


def _preprocess(x, edge_index, W1, b1, W2, b2):
    """Host-side graph preprocessing + input staging for all 8 cores."""
    src = np.concatenate([np.asarray(edge_index[0]), np.arange(N, dtype=np.int64)])
    dst = np.concatenate([np.asarray(edge_index[1]), np.arange(N, dtype=np.int64)])

    deg = np.bincount(dst, minlength=N).astype(np.float64)
    dinv = np.where(deg > 0, 1.0 / np.sqrt(deg), 0.0).astype(np.float32)

    rows = _rowid(src)
    core = dst // SH
    dloc = dst - core * SH
    tile_e = dloc // P
    dlane = dloc % P
    lo = rows < LOWROWS

    # layer-2 source addressing: compact cc_outA (AllGather pieces 0-2, src
    # tiles < PB[3]) and cc_outB (piece 3) — collectives demand contiguous
    # in/out, and the A/B split lets layer-2 A-gathers start while piece 3's
    # collective is still in flight.  Each node's h2' row is one 32B unit; a
    # 256B gather descriptor covers 8 consecutive units, the wanted one
    # selected by a per-chunk-uniform rhs sub-slice (edges sorted by
    # row32 % 8 within each (tile, src-group)).
    wt = np.array([PB[k + 1] - PB[k] for k in range(4)], dtype=np.int64)
    offA32 = np.concatenate([[0], np.cumsum(NCORES * P * wt[:3])])
    piece_of = np.zeros(NT, dtype=np.int64)
    for k in range(4):
        piece_of[PB[k] : PB[k + 1]] = k
    sc = src // SH
    sr = src - sc * SH
    st = sr // P
    sp = sr % P
    sk = piece_of[st]
    grpB = sk == 3
    row32 = np.where(
        grpB,
        (sc * P + sp) * wt[3] + (st - PB[3]),
        offA32[np.minimum(sk, 2)] + (sc * P + sp) * wt[np.minimum(sk, 2)]
        + (st - np.array(PB)[np.minimum(sk, 2)]),
    )
    idx2 = row32 // 8
    sub2 = row32 % 8

    # per-(core, pass, tile[, sub]) edge counts -> static lane starts.
    # L1 passes: low/high rows.  L2 passes: src group A (pieces 0-2) / B.
    cnt = np.zeros((NCORES, 2, NT), dtype=np.int64)
    cnt2 = np.zeros((NCORES, 2, NT, 8), dtype=np.int64)
    for c in range(NCORES):
        for pi, m in ((0, (core == c) & lo), (1, (core == c) & ~lo)):
            cnt[c, pi] = np.bincount(tile_e[m], minlength=NT)
        for gi, m in ((0, (core == c) & ~grpB), (1, (core == c) & grpB)):
            np.add.at(cnt2[c, gi], (tile_e[m], sub2[m]), 1)
    maxcnt = cnt.max(axis=0)                          # [2, NT]
    S = np.zeros((2, NT + 1), dtype=np.int64)
    S[:, 1:] = np.cumsum(maxcnt, axis=1)
    nchunks = [int((S[pi, -1] + P - 1) // P) for pi in (0, 1)]
    CL, CH = nchunks
    maxcnt2 = cnt2.max(axis=0).reshape(2, NT * 8)     # [grp, (tile, sub)]
    S2 = np.zeros((2, NT * 8 + 1), dtype=np.int64)
    S2[:, 1:] = np.cumsum(maxcnt2, axis=1)
    nchunks2 = [int((S2[gi, -1] + P - 1) // P) for gi in (0, 1)]
    CA2, CB2 = nchunks2

    # static matmul schedules: per tile -> [(pass, chunk, mcol, rhs_off)]
    tile_chunks = [[] for _ in range(NT)]
    mcol_map = [np.full((nchunks[pi], NT), -1, dtype=np.int64) for pi in (0, 1)]
    mcol_map2 = [
        np.full((nchunks2[gi], NT * 8), -1, dtype=np.int64) for gi in (0, 1)
    ]
    mcol = 0
    for t in range(NT):
        for pi in (0, 1):
            if maxcnt[pi, t] == 0:
                continue
            c0, c1 = int(S[pi, t] // P), int((S[pi, t + 1] - 1) // P)
            for c in range(c0, c1 + 1):
                tile_chunks[t].append((pi, c, mcol, 0))
                mcol_map[pi][c, t] = mcol
                mcol += 1
    tile_chunks2 = [[] for _ in range(NT)]
    for t in range(NT):
        for gi in (0, 1):
            for j in range(8):
                g = t * 8 + j
                if maxcnt2[gi, g] == 0:
                    continue
                c0 = int(S2[gi, g] // P)
                c1 = int((S2[gi, g + 1] - 1) // P)
                for c in range(c0, c1 + 1):
                    tile_chunks2[t].append((gi, c, mcol, j * FO))
                    mcol_map2[gi][c, g] = mcol
                    mcol += 1
    M_total = mcol

    # per-core streams: gather idx tables + selection values
    idx16 = []
    selv = []
    for c in range(NCORES):
        iv = np.zeros((CL + CH + CA2 + CB2) * P, dtype=np.int64)
        sv = np.full((P, M_total), -1.0, dtype=np.float32)
        for pi, m in ((0, (core == c) & lo), (1, (core == c) & ~lo)):
            te = tile_e[m]
            order = np.argsort(te, kind="stable")
            te_s = te[order]
            rows_s = rows[m][order] - (0 if pi == 0 else LOWROWS)
            dl_s = dlane[m][order]
            starts = np.searchsorted(te_s, np.arange(NT))
            pos = S[pi][te_s] + (np.arange(te_s.size) - starts[te_s])
            base = 0 if pi == 0 else CL * P
            iv[base + pos] = rows_s
            ch = pos // P
            mc = mcol_map[pi][ch, te_s]
            assert (mc >= 0).all()
            sv[pos % P, mc] = dl_s.astype(np.float32)
        # L2 streams: per src group, sorted by (tile, sub)
        for gi, m in ((0, (core == c) & ~grpB), (1, (core == c) & grpB)):
            g_e = tile_e[m] * 8 + sub2[m]
            order = np.argsort(g_e, kind="stable")
            g_s = g_e[order]
            idx2_s = idx2[m][order]
            dl_s = dlane[m][order]
            starts = np.searchsorted(g_s, np.arange(NT * 8))
            pos = S2[gi][g_s] + (np.arange(g_s.size) - starts[g_s])
            base = (CL + CH + (0 if gi == 0 else CA2)) * P
            iv[base + pos] = idx2_s
            mc = mcol_map2[gi][pos // P, g_s]
            assert (mc >= 0).all()
            sv[pos % P, mc] = dl_s.astype(np.float32)
        idx16.append(_pack_idx16(iv))
        selv.append(sv)

    # xpack [128, XCOLS] bf16: partition (a*64+f), col j <-> xs[a*XCOLS+j, f]
    xs = np.zeros((NPAD, F), dtype=np.float32)
    xs[:N] = np.asarray(x, dtype=np.float32) * dinv[:, None]
    xpack = xs.reshape(2, XCOLS, F).transpose(0, 2, 1).reshape(P, XCOLS).astype(BF16)

    dinv_s = np.zeros((NCORES, P, NT), dtype=np.float32)
    for c in range(NCORES):
        ll = np.arange(min(SH, N - c * SH))
        dinv_s[c, ll % P, ll // P] = dinv[c * SH : c * SH + ll.size]

    b1z = not np.any(np.asarray(b1))
    b2z = not np.any(np.asarray(b2))
    common = {
        "xpack": xpack,
        "w1": np.concatenate([np.asarray(W1, np.float32)] * 2, axis=0).astype(BF16),
        "w2": np.asarray(W2, dtype=np.float32).astype(BF16),
    }
    if not b1z:
        common["b1r"] = np.broadcast_to(np.asarray(b1, np.float32), (P, F)).copy()
    if not b2z:
        common["b2r"] = np.broadcast_to(np.asarray(b2, np.float32), (P, FO)).copy()
    in_maps = []
    for c in range(NCORES):
        m = dict(common)
        m["dinvs"] = dinv_s[c]
        m["idx16"] = idx16[c]
        m["selv"] = selv[c]
        in_maps.append(m)

    static = {
        "CL": CL,
        "CH": CH,
        "CA2": CA2,
        "CB2": CB2,
        "M": M_total,
        "tile_chunks": tile_chunks,
        "tile_chunks2": tile_chunks2,
        "b1z": b1z,
        "b2z": b2z,
    }
    return in_maps, static


_WAIT_LIMIT = int(os.environ.get("GCN_WAIT_LIMIT", "1"))


def _legalize_waits(nc, limit=None):
    """Split >limit semaphore waits into standalone NOPs on the same engine.

    Walrus codegen rejects instructions whose sync_info carries more wait
    conditions than the ISA sync fields hold ("Too many sync wait commands").
    A chain of no-ops each carrying <=limit waits is semantically identical
    (waits are AND conditions and the engine queue is in-order).
    """
    if limit is None:
        limit = _WAIT_LIMIT
    import bass_rust as _br

    uid = 0
    for fn in nc.m.functions:
        for bb in fn.blocks:
            out = []
            changed = False
            for ins in bb.instructions:
                si = ins.sync_info
                if si is not None and len(si.on_wait) > limit:
                    waits = list(si.on_wait)
                    excess, keep = waits[:-limit], waits[-limit:]
                    for i in range(0, len(excess), limit):
                        nop = mybir.InstNoOp(name=f"waitsplit_{uid}", ins=[], outs=[])
                        uid += 1
                        nop.engine = ins.engine
                        nop.sync_info = _br.SyncInfo(
                            on_wait=excess[i : i + limit], on_update=[]
                        )
                        out.append(nop)
                    ins.sync_info = _br.SyncInfo(
                        on_wait=keep, on_update=list(si.on_update)
                    )
                    changed = True
                out.append(ins)
            if changed:
                bb.instructions = out
    return nc


def _build(st):
    dt = mybir.dt
    Alu = mybir.AluOpType
    Act = mybir.ActivationFunctionType
    CL, CH, CA2, CB2 = st["CL"], st["CH"], st["CA2"], st["CB2"]
    M = st["M"]
    tile_chunks = st["tile_chunks"]
    tile_chunks2 = st["tile_chunks2"]
    b1z, b2z = st["b1z"], st["b2z"]

    nc = bass.Bass(num_devices=NCORES)

    # ---- I/O ----
    xpack_e = nc.dram_tensor("xpack", [P, XCOLS], dt.bfloat16, kind="ExternalInput")
    w1_e = nc.dram_tensor("w1", [2 * F, F], dt.bfloat16, kind="ExternalInput")
    w2_e = nc.dram_tensor("w2", [F, FO], dt.bfloat16, kind="ExternalInput")
    if not b1z:
        b1_e = nc.dram_tensor("b1r", [P, F], dt.float32, kind="ExternalInput")
    if not b2z:
        b2_e = nc.dram_tensor("b2r", [P, FO], dt.float32, kind="ExternalInput")
    dinvs_e = nc.dram_tensor("dinvs", [P, NT], dt.float32, kind="ExternalInput")
    idx16_e = nc.dram_tensor(
        "idx16", [P, (CL + CH + CA2 + CB2) * 8], dt.int16, kind="ExternalInput"
    )
    selv_e = nc.dram_tensor("selv", [P, M], dt.float32, kind="ExternalInput")
    out_e = nc.dram_tensor("out", [P, NT * FO], dt.float32, kind="ExternalOutput")

    # ---- internal DRAM ----
    # h_dram row (p*8+c)*49+t = 256B-padded h' row of node c*6272+t*128+p.
    # cc_out is COMPACT [50176, 16]: 4 contiguous AllGather piece regions
    # (collectives demand contiguous in/out); layer 2 gathers 256B = 8 rows
    # per descriptor, the wanted 32B row picked by a per-chunk rhs sub-slice.
    h_dram = nc.dram_tensor("h_dram", [NPAD, EPR], dt.bfloat16)
    cc_ins = [
        nc.dram_tensor(
            f"cc_in{k}", [P, (PB[k + 1] - PB[k]) * FO], dt.bfloat16
        )
        for k in range(4)
    ]
    NRA = NCORES * P * PB[3]            # compact rows in cc_outA (pieces 0-2)
    NRB = NCORES * P * (NT - PB[3])     # compact rows in cc_outB (piece 3)
    cc_outA = nc.dram_tensor(
        "cc_outA", [NRA, FO], dt.bfloat16, addr_space="Shared"
    )
    cc_outB = nc.dram_tensor(
        "cc_outB", [NRB, FO], dt.bfloat16, addr_space="Shared"
    )

    with tile.TileContext(nc) as tc:
        with tc.tile_pool(name="const", bufs=1) as cp:
            w1 = cp.tile([2 * F, F], dt.bfloat16, tag="w1")
            nc.sync.dma_start(out=w1[:], in_=w1_e[:, :])
            w2 = cp.tile([F, FO], dt.bfloat16, tag="w2")
            nc.sync.dma_start(out=w2[:], in_=w2_e[:, :])
            if not b1z:
                b1r = cp.tile([P, F], dt.float32, tag="b1r")
                nc.sync.dma_start(out=b1r[:], in_=b1_e[:, :])
            if not b2z:
                b2r = cp.tile([P, FO], dt.float32, tag="b2r")
                nc.sync.dma_start(out=b2r[:], in_=b2_e[:, :])
            dinvs = cp.tile([P, NT], dt.float32, tag="dinvs")
            nc.sync.dma_start(out=dinvs[:], in_=dinvs_e[:, :])
            idx16 = cp.tile(
                [P, (CL + CH + CA2 + CB2) * 8], dt.int16, tag="idx16"
            )
            nc.sync.dma_start(out=idx16[:], in_=idx16_e[:, :])
            selv = cp.tile([P, M], dt.float32, tag="selv")
            nc.sync.dma_start(out=selv[:], in_=selv_e[:, :])
            iota = cp.tile([P, P], dt.bfloat16, tag="iota")
            nc.gpsimd.iota(
                iota[:],
                pattern=[[1, P]],
                base=0,
                channel_multiplier=0,
                allow_small_or_imprecise_dtypes=True,
            )
            ident = cp.tile([P, P], dt.bfloat16, tag="ident")
            make_identity(nc, ident[:])
            out1 = cp.tile([P, NT * F], dt.bfloat16, tag="out1")
            h2st = cp.tile([P, NT * FO], dt.bfloat16, tag="h2st")
            outst = cp.tile([P, NT * FO], dt.float32, tag="outst")

            tc.strict_bb_all_engine_barrier()
            # dma_gather lives in the Q7 "mlp" extended-instruction library.
            # bass's pseudo reload ships with an empty instr payload, which
            # walrus rejects ("ISA wrong length") — fill the 64B struct.
            _li = nc.gpsimd.load_library(library_config.mlp)
            _instr, _fx = bass_isa.isa_struct(
                nc.isa,
                nc.isa.Opcode.NEURON_ISA_TPB_OPCODE_PSEUDO_INST,
                {"pseudo_opcode": 2, "lib_index": library_config.mlp.index},
                struct_name="NEURON_ISA_TPB_PSEUDO_LIBRARY_RELOAD_INDEX_STRUCT",
            )
            _li.ins.instr = _instr

            # ------------- Phase A: h' = bf16((dinv*x) @ W1) -----------------
            XB = 14 * P  # node tiles per xpack block (14 tiles, 2 psum groups)
            with (
                tc.tile_pool(name="xa", bufs=3) as xpool,
                tc.tile_pool(name="ha", bufs=6) as hpool,
                tc.tile_pool(name="pha", bufs=6, space="PSUM") as phpool,
            ):
                gi = 0
                for blk in range(0, XCOLS, XB):
                    xb = xpool.tile([P, XB], dt.bfloat16, tag="xb")
                    nc.sync.dma_start(out=xb[:], in_=xpack_e[:, blk : blk + XB])
                    for a in (0, 1):
                        for g0 in range(0, XB // P, TPG):
                            ph = phpool.tile([P, TPG * F], dt.float32, tag="ph")
                            for i in range(TPG):
                                m = g0 + i
                                nc.tensor.matmul(
                                    out=ph[:, i * F : (i + 1) * F],
                                    lhsT=xb[
                                        a * F : (a + 1) * F, m * P : (m + 1) * P
                                    ],
                                    rhs=w1[a * F : (a + 1) * F, :],
                                    start=True,
                                    stop=True,
                                )
                            hst = hpool.tile([P, TPG * F], dt.bfloat16, tag="hst")
                            # psum->sbuf bf16 casts on DVE (idle in phase A);
                            # writes alternate between the Act and SP queues
                            # so no single SEQ paces the pipeline
                            nc.vector.tensor_copy(out=hst[:], in_=ph[:])
                            tg0 = a * XH + blk // P + g0
                            c_, t0 = tg0 // NT, tg0 % NT
                            dst = bass.AP(
                                h_dram,
                                (c_ * NT + t0) * EPR,
                                [[NCORES * NT * EPR, P], [EPR, TPG], [1, F]],
                            )
                            eng = nc.scalar if gi % 2 == 0 else nc.sync
                            eng.dma_start(out=dst, in_=hst[:])
                            gi += 1

            # ------------- shared aggregation machinery ----------------------
            nidx_regs = {}
            cc_delay = []  # [BassInstruction, remaining gather-blocks]

            def aggregation(passes, tile_entries, fw, psum_pool,
                            gpool, selpool, tailpool, pool_every=0):
                # passes: per gather pass (nchunks, src_ap, idx chunk base)
                gbufs = {}

                def blocks_for(nch):
                    # full G-chunk blocks, but a tail of small blocks so the
                    # end-of-pass pipeline drain is short
                    bl, c0 = [], 0
                    while c0 < nch:
                        w = G if nch - c0 > 40 else min(8, nch - c0)
                        bl.append((c0, w))
                        c0 += w
                    return bl

                blocks = [blocks_for(p[0]) for p in passes]
                blk_of = []
                for pi, p in enumerate(passes):
                    m_ = np.zeros(p[0], dtype=np.int64)
                    for bi, (c0, w) in enumerate(blocks[pi]):
                        m_[c0 : c0 + w] = bi
                    blk_of.append(m_)

                def issue(pi, b):
                    nch, src, base = passes[pi]
                    c0, w = blocks[pi][b]
                    gb = gpool.tile([P, w * EPR], dt.bfloat16, tag=f"gb{w}")
                    # one shared Pool register per distinct idx count (a fresh
                    # to_reg per gather exhausts the register file)
                    if w * P not in nidx_regs:
                        nidx_regs[w * P] = nc.gpsimd.to_reg(w * P)
                    gin = nc.gpsimd.dma_gather(
                        out_ap=gb[:, : w * EPR].rearrange("p (s e) -> p s e", e=EPR),
                        in_ap=src,
                        idxs_ap=idx16[:, (base + c0) * 8 : (base + c0 + w) * 8],
                        num_idxs=w * P,
                        num_idxs_reg=nidx_regs[w * P],
                        elem_size=EPR,
                        single_packet=False,
                    )
                    gbufs[(pi, b)] = gb
                    # attach any pending collective so it lands in Pool's
                    # stream only after enough gather desc-gen has been
                    # issued (its cc_in wait is then already satisfied)
                    for ent in cc_delay:
                        ent[1] -= 1
                        if ent[1] == 0:
                            add_dep_helper(
                                ent[0].ins, gin.ins, sync=False,
                                reason="delay collective past gathers",
                            )
                    cc_delay[:] = [e for e in cc_delay if e[1] > 0]

                nsel = [0]

                def build_sel(pool, mc, allow_pool=False):
                    # offload a fraction of builds to the gpsimd engine when
                    # the layer is DVE-bound
                    sel = pool.tile([P, P], dt.bfloat16, tag="sel")
                    nsel[0] += 1
                    eng = (
                        nc.gpsimd
                        if allow_pool and pool_every and nsel[0] % pool_every == 0
                        else nc.vector
                    )
                    eng.tensor_scalar(
                        out=sel[:],
                        in0=iota[:],
                        scalar1=selv[:, mc : mc + 1],
                        scalar2=None,
                        op0=Alu.is_equal,
                    )
                    return sel

                # prebuild the last tiles' selection matrices up front: at
                # drain time DVE's in-order queue is otherwise still full of
                # them (blocked behind per-tile epilogue reduces)
                tail_sels = {}
                for t in range(NT - 6, NT):
                    for ent in sorted(tile_entries[t], key=lambda e: -e[0]):
                        if len(tail_sels) < 120:
                            tail_sels[ent[2]] = build_sel(tailpool, ent[2])

                def sweep(entries_of, epilogue, tag):
                    for t in range(NT):
                        ents = entries_of(t)
                        if not ents:
                            epilogue(t, None)
                            continue
                        pt = psum_pool.tile([P, fw], dt.float32, tag=tag)
                        nmm = len(ents)
                        for k, (pi, c, mc, roff) in enumerate(ents):
                            b = int(blk_of[pi][c])
                            if (pi, b) not in gbufs:
                                issue(pi, b)
                            c0b = blocks[pi][b][0]
                            rhs = gbufs[(pi, b)][
                                :,
                                (c - c0b) * EPR + roff :
                                (c - c0b) * EPR + roff + fw,
                            ]
                            sel = tail_sels.get(mc)
                            if sel is None:
                                # pool-built sels trail Pool's ring-gated
                                # desc-gen, so only offload early entries
                                early = c < (passes[pi][0] * 3) // 5
                                sel = build_sel(selpool, mc, allow_pool=early)
                            nc.tensor.matmul(
                                out=pt[:],
                                lhsT=sel[:],
                                rhs=rhs,
                                start=(k == 0),
                                stop=(k == nmm - 1),
                            )
                        epilogue(t, pt)

                return sweep

            # ------------- Layer 1 aggregation + transform + collective ------
            ndone = [0]
            with (
                tc.tile_pool(name="gb1", bufs=5) as gpool1,
                tc.tile_pool(name="sel1", bufs=64) as selpool1,
                tc.tile_pool(name="selt1", bufs=128) as tailpool1,
                tc.tile_pool(name="pagg1", bufs=6, space="PSUM") as pp1,
                tc.tile_pool(name="ptr", bufs=1, space="PSUM") as ptrpool,
                tc.tile_pool(name="ph2", bufs=1, space="PSUM") as ph2pool,
                tc.tile_pool(name="o1t", bufs=2) as o1tpool,
                tc.tile_pool(name="ep1", bufs=2) as ep1pool,
            ):

                def fire_piece(k):
                    # AllGather piece (Pool engine — the only one walrus
                    # allows; in/out must be contiguous).  Piece k's output
                    # is a contiguous region of the compact cc_out.  An
                    # artificial dep pushes the instruction ~7 gather blocks
                    # past its milestone in Pool's stream so its cc_in wait
                    # doesn't stall desc-gen.
                    t0, t1 = PB[k], PB[k + 1]
                    w = (t1 - t0) * FO
                    cct = cc_outA if k < 3 else cc_outB
                    off = 0 if k == 3 else NCORES * P * FO * t0
                    nc.sync.dma_start(
                        out=cc_ins[k][:, :], in_=h2st[:, t0 * FO : t1 * FO]
                    )
                    cc = nc.gpsimd.collective_compute(
                        "AllGather",
                        mybir.AluOpType.bypass,
                        replica_groups=[list(range(NCORES))],
                        ins=[cc_ins[k][:, :]],
                        outs=[
                            bass.AP(
                                cct,
                                off,
                                [[P * w, NCORES], [w, P], [1, w]],
                            )
                        ],
                    )
                    if k < 3:
                        cc_delay.append([cc, 7])

                def on_stop1(t, pt):
                    # epilogue: out1 = relu(dinv_d * sum (+ b1))
                    if b1z:
                        nc.scalar.activation(
                            out=out1[:, t * F : (t + 1) * F],
                            in_=pt[:],
                            func=Act.Relu,
                            scale=dinvs[:, t : t + 1],
                        )
                    else:
                        tmp = ep1pool.tile([P, F], dt.float32, tag="tmp")
                        nc.vector.tensor_scalar(
                            out=tmp[:],
                            in0=pt[:],
                            scalar1=dinvs[:, t : t + 1],
                            scalar2=None,
                            op0=Alu.mult,
                        )
                        nc.vector.tensor_tensor(
                            out=tmp[:], in0=tmp[:], in1=b1r[:], op=Alu.add
                        )
                        nc.scalar.activation(
                            out=out1[:, t * F : (t + 1) * F],
                            in_=tmp[:],
                            func=Act.Relu,
                        )
                    # layer-2 transform for this tile: h2' = dinv_d*(out1@W2)
                    ptr_ = ptrpool.tile([P, P], dt.bfloat16, tag="ptr")
                    nc.tensor.transpose(
                        out=ptr_[:F, :],
                        in_=out1[:, t * F : (t + 1) * F],
                        identity=ident[:],
                    )
                    o1T = o1tpool.tile([F, P], dt.bfloat16, tag="o1T")
                    nc.vector.tensor_copy(out=o1T[:], in_=ptr_[:F, :])
                    ph2 = ph2pool.tile([P, FO], dt.float32, tag="ph2")
                    nc.tensor.matmul(
                        out=ph2[:], lhsT=o1T[:], rhs=w2[:, :], start=True, stop=True
                    )
                    nc.scalar.activation(
                        out=h2st[:, t * FO : (t + 1) * FO],
                        in_=ph2[:],
                        func=Act.Copy,
                        scale=dinvs[:, t : t + 1],
                    )
                    ndone[0] += 1
                    for k in range(4):
                        if ndone[0] == PB[k + 1]:
                            fire_piece(k)

                passes1 = [
                    (CL, h_dram.ap(), 0),
                    (
                        CH,
                        bass.AP(
                            h_dram,
                            LOWROWS * EPR,
                            [[EPR, NPAD - LOWROWS], [1, EPR]],
                        ),
                        CL,
                    ),
                ]
                sweep1 = aggregation(passes1, tile_chunks, F, pp1,
                                     gpool1, selpool1, tailpool1)
                sweep1(lambda t: tile_chunks[t], on_stop1, "agg")

            # ------------- Layer 2 aggregation + log_softmax -----------------
            with (
                tc.tile_pool(name="gb2", bufs=5) as gpool2,
                tc.tile_pool(name="sel2", bufs=192) as selpool2,
                tc.tile_pool(name="selt2", bufs=128) as tailpool2,
                tc.tile_pool(name="pagg2", bufs=6, space="PSUM") as pp2,
                tc.tile_pool(name="ep2", bufs=8) as ep2pool,
            ):

                accA = cp.tile([P, NT * FO], dt.float32, tag="accA")
                accB = cp.tile([P, NT * FO], dt.float32, tag="accB")

                def flush_out(t):
                    # flush finished outst columns piecewise so the final
                    # write isn't one big end-serialized DMA
                    for q0, q1 in ((0, 13), (13, 26), (26, 38), (38, 49)):
                        if t == q1 - 1:
                            nc.sync.dma_start(
                                out=out_e[:, q0 * FO : q1 * FO],
                                in_=outst[:, q0 * FO : q1 * FO],
                            )

                def spill_to(acc):
                    # spill partial sums (pre-scaled by dinv_d) on Act, so
                    # DVE's in-order queue carries ONLY sel builds during the
                    # gather sweeps (no head-of-line epilogue waits)
                    def spill(t, pt):
                        if pt is None:
                            nc.gpsimd.memset(acc[:, t * FO : (t + 1) * FO], 0.0)
                            return
                        nc.scalar.activation(
                            out=acc[:, t * FO : (t + 1) * FO],
                            in_=pt[:],
                            func=Act.Copy,
                            scale=dinvs[:, t : t + 1],
                        )
                    return spill

                def epi_final(t):
                    # log_softmax from the two spilled partials
                    tmp = ep2pool.tile([P, FO], dt.float32, tag="tmp2")
                    nc.vector.tensor_tensor(
                        out=tmp[:],
                        in0=accA[:, t * FO : (t + 1) * FO],
                        in1=accB[:, t * FO : (t + 1) * FO],
                        op=Alu.add,
                    )
                    if not b2z:
                        nc.vector.tensor_tensor(
                            out=tmp[:], in0=tmp[:], in1=b2r[:], op=Alu.add
                        )
                    mx = ep2pool.tile([P, 1], dt.float32, tag="mx")
                    nc.vector.reduce_max(
                        out=mx[:], in_=tmp[:], axis=mybir.AxisListType.X, negate=True
                    )
                    ex = ep2pool.tile([P, FO], dt.float32, tag="ex")
                    nc.scalar.activation(
                        out=ex[:], in_=tmp[:], func=Act.Exp, bias=mx[:, 0:1]
                    )
                    sm = ep2pool.tile([P, 1], dt.float32, tag="sm")
                    nc.vector.reduce_sum(
                        out=sm[:], in_=ex[:], axis=mybir.AxisListType.X
                    )
                    lg = ep2pool.tile([P, 1], dt.float32, tag="lg")
                    nc.scalar.activation(out=lg[:], in_=sm[:], func=Act.Ln)
                    nc.vector.tensor_scalar(
                        out=outst[:, t * FO : (t + 1) * FO],
                        in0=tmp[:],
                        scalar1=mx[:, 0:1],
                        scalar2=lg[:, 0:1],
                        op0=Alu.add,
                        op1=Alu.subtract,
                    )
                    flush_out(t)

                passes2 = [
                    (
                        CA2,
                        bass.AP(cc_outA, 0, [[EPR, NRA * FO // EPR], [1, EPR]]),
                        CL + CH,
                    ),
                    (
                        CB2,
                        bass.AP(cc_outB, 0, [[EPR, NRB * FO // EPR], [1, EPR]]),
                        CL + CH + CA2,
                    ),
                ]
                sweep2 = aggregation(passes2, tile_chunks2, FO, pp2,
                                     gpool2, selpool2, tailpool2,
                                     pool_every=5)
                # sweep A first (src pieces 0-2, available before the last
                # collective lands), then sweep B, then the epilogues
                sweep2(
                    lambda t: [e for e in tile_chunks2[t] if e[0] == 0],
                    spill_to(accA), "agg",
                )
                sweep2(
                    lambda t: [e for e in tile_chunks2[t] if e[0] == 1],
                    spill_to(accB), "agg",
                )
                for t in range(NT):
                    epi_final(t)

    _legalize_waits(nc)
    return nc


def kernel(x, edge_index, W1, b1, W2, b2, _trace=False, _trace_kwargs=None):
    in_maps, st = _preprocess(x, edge_index, W1, b1, W2, b2)
    key = (
        st["CL"],
        st["CH"],
        st["CA2"],
        st["CB2"],
        st["M"],
        st["b1z"],
        st["b2z"],
        tuple(tuple(tc_) for tc_ in map(tuple, st["tile_chunks"])),
        tuple(tuple(tc_) for tc_ in map(tuple, st["tile_chunks2"])),
    )
    if key not in _CACHE:
        _CACHE[key] = _build(st)
    nc = _CACHE[key]

    res = run_bass_kernel_spmd(
        nc,
        in_maps,
        core_ids=list(range(NCORES)),
        trace=_trace,
        **(_trace_kwargs or {}),
    )
    out = np.empty((N, FO), dtype=np.float32)
    for c in range(NCORES):
        o = np.asarray(res.results[c]["out"], dtype=np.float32)
        o = o.reshape(P, NT, FO).transpose(1, 0, 2).reshape(NT * P, FO)
        k = min(SH, N - c * SH)
        out[c * SH : c * SH + k] = o[:k]
    kernel._last_result = res
    return out


# revision 93
# speedup vs baseline: 1.3288x; 1.0042x over previous
"""GCN (2-layer, PyG GCNConv-style) on 8 Trainium2 NeuronCores.

Strategy:
  - Destination nodes sharded 8 ways (6272 dst slots / core); edges
    partitioned by destination shard on host.  Weights replicated; every
    core computes the full first-layer transform h' = (dinv*x) @ W1 (dinv
    pre-folded on host) so per-edge gathers are purely local.
  - Per-edge rows fetched with dma_gather (int16 indices, 256B descs);
    layer 1 reads padded h_dram rows (row (p*8+c)*49+t, int16 range forces
    a low/high two-pass split); layer 2 reads the COMPACT AllGather output
    (8 nodes' h2' per 256B desc, the wanted 32B row picked by a per-chunk
    rhs sub-slice, edges pre-sorted by row%8 — no expand step, idx fits
    int16 in one pass).
  - Aggregation: edge streams lane-packed with static per-(tile[,sub])
    starts (max count across cores); chunks straddling group boundaries
    issue one matmul per group with a lane-masked one-hot selection
    matrix (built on DVE, 1-in-5 on gpsimd) as matmul lhsT, accumulating
    per-dst-tile in PSUM.  Per-tile epilogues fold the dst-side dinv into
    a single Activation op (relu for layer 1).
  - Layer-2 transform h2' = dinv*(out1 @ W2) runs per-tile right after
    that tile's epilogue; the [6272,16] shard is AllGathered in 4
    contiguous pieces (Pool engine, delayed in-stream by artificial deps)
    overlapped with the layer-1 aggregation; layer-2 gathers run in two
    sweeps (srcs from pieces 0-2 first, hiding piece 3's collective),
    spilling dinv-scaled partials, with a DVE-pure log_softmax epilogue
    sweep at the end.
  - Output sharded by dst, flushed piecewise; host concatenates.
"""

import os
import sys

import numpy as np

for _p in ("/opt/trn_rl_repo", "/root/.axon_site/_ro/trn_rl_repo"):
    if os.path.isdir(_p) and _p not in sys.path:
        sys.path.insert(0, _p)

import ml_dtypes  # noqa: E402
import concourse.bass as bass  # noqa: E402
import concourse.mybir as mybir  # noqa: E402
import concourse.tile as tile  # noqa: E402
from concourse.tile_rust import add_dep_helper  # noqa: E402
from concourse.bass_utils import run_bass_kernel_spmd  # noqa: E402
from concourse.masks import make_identity  # noqa: E402
from concourse import library_config  # noqa: E402
import concourse.bass_isa as bass_isa  # noqa: E402

# ---------------- static problem config (hardcoded per contract) -------------
N = 50000
E = 800000
F = 64          # F_IN == F_HID
FO = 16         # F_OUT
NCORES = 8
P = 128
NT = 49                   # dst tiles per core
SH = NT * P               # 6272 dst slots per core (last core: 6096 real)
NTH = 392                 # node tiles (50176 = 392*128 padded nodes)
NPAD = NTH * P            # 50176
XH = NTH // 2             # 196 node tiles per half
XCOLS = XH * P            # 25088
G = 32                    # chunks per dma_gather instruction (4096 edges)
EPR = 128                 # padded row length (elements, bf16) = 256 bytes
LOWROWS = 32768           # int16 index range split
TPG = 7                   # node tiles per phase-A psum group
PB = [0, 10, 20, 31, 49]  # collective piece tile boundaries

BF16 = ml_dtypes.bfloat16

_CACHE = {}


def _rowid(n):
    # node n = c*SH + t*128 + p  ->  row (p*8+c)*49 + t; matches both the
    # phase-A h_dram write pattern and the padded cc_out AllGather layout
    # (rank innermost keeps self-loops balanced across the int16 lo/hi cut)
    c, r = n // SH, n % SH
    return ((r % P) * NCORES + c) * NT + r // P


def _pack_idx16(vals):
    """Edge-position-ordered values [C*128] -> int16 idx table [128, C*8].

    dma_gather reads index k from (partition k%16, col k//16), replicated
    across the 8 q7 cores (partition stripes of 16).
    """
    k = np.arange(vals.size)
    tbl = np.zeros((16, vals.size // 16), dtype=np.int16)
    tbl[k % 16, k // 16] = vals.astype(np.int16)
    return np.tile(tbl, (8, 1))


def _preprocess(x, edge_index, W1, b1, W2, b2):
    """Host-side graph preprocessing + input staging for all 8 cores."""
    src = np.concatenate([np.asarray(edge_index[0]), np.arange(N, dtype=np.int64)])
    dst = np.concatenate([np.asarray(edge_index[1]), np.arange(N, dtype=np.int64)])

    deg = np.bincount(dst, minlength=N).astype(np.float64)
    dinv = np.where(deg > 0, 1.0 / np.sqrt(deg), 0.0).astype(np.float32)

    rows = _rowid(src)
    core = dst // SH
    dloc = dst - core * SH
    tile_e = dloc // P
    dlane = dloc % P
    lo = rows < LOWROWS

    # layer-2 source addressing: compact cc_outA (AllGather pieces 0-2, src
    # tiles < PB[3]) and cc_outB (piece 3) — collectives demand contiguous
    # in/out, and the A/B split lets layer-2 A-gathers start while piece 3's
    # collective is still in flight.  Each node's h2' row is one 32B unit; a
    # 256B gather descriptor covers 8 consecutive units, the wanted one
    # selected by a per-chunk-uniform rhs sub-slice (edges sorted by
    # row32 % 8 within each (tile, src-group)).
    wt = np.array([PB[k + 1] - PB[k] for k in range(4)], dtype=np.int64)
    offA32 = np.concatenate([[0], np.cumsum(NCORES * P * wt[:3])])
    piece_of = np.zeros(NT, dtype=np.int64)
    for k in range(4):
        piece_of[PB[k] : PB[k + 1]] = k
    sc = src // SH
    sr = src - sc * SH
    st = sr // P
    sp = sr % P
    sk = piece_of[st]
    grpB = sk == 3
    row32 = np.where(
        grpB,
        (sc * P + sp) * wt[3] + (st - PB[3]),
        offA32[np.minimum(sk, 2)] + (sc * P + sp) * wt[np.minimum(sk, 2)]
        + (st - np.array(PB)[np.minimum(sk, 2)]),
    )
    idx2 = row32 // 8
    sub2 = row32 % 8

    # per-(core, pass, tile[, sub]) edge counts -> static lane starts.
    # L1 passes: low/high rows.  L2 passes: src group A (pieces 0-2) / B.
    cnt = np.zeros((NCORES, 2, NT), dtype=np.int64)
    cnt2 = np.zeros((NCORES, 2, NT, 8), dtype=np.int64)
    for c in range(NCORES):
        for pi, m in ((0, (core == c) & lo), (1, (core == c) & ~lo)):
            cnt[c, pi] = np.bincount(tile_e[m], minlength=NT)
        for gi, m in ((0, (core == c) & ~grpB), (1, (core == c) & grpB)):
            np.add.at(cnt2[c, gi], (tile_e[m], sub2[m]), 1)
    maxcnt = cnt.max(axis=0)                          # [2, NT]
    S = np.zeros((2, NT + 1), dtype=np.int64)
    S[:, 1:] = np.cumsum(maxcnt, axis=1)
    nchunks = [int((S[pi, -1] + P - 1) // P) for pi in (0, 1)]
    CL, CH = nchunks
    maxcnt2 = cnt2.max(axis=0).reshape(2, NT * 8)     # [grp, (tile, sub)]
    S2 = np.zeros((2, NT * 8 + 1), dtype=np.int64)
    S2[:, 1:] = np.cumsum(maxcnt2, axis=1)
    nchunks2 = [int((S2[gi, -1] + P - 1) // P) for gi in (0, 1)]
    CA2, CB2 = nchunks2

    # static matmul schedules: per tile -> [(pass, chunk, mcol, rhs_off)]
    tile_chunks = [[] for _ in range(NT)]
    mcol_map = [np.full((nchunks[pi], NT), -1, dtype=np.int64) for pi in (0, 1)]
    mcol_map2 = [
        np.full((nchunks2[gi], NT * 8), -1, dtype=np.int64) for gi in (0, 1)
    ]
    mcol = 0
    for t in range(NT):
        for pi in (0, 1):
            if maxcnt[pi, t] == 0:
                continue
            c0, c1 = int(S[pi, t] // P), int((S[pi, t + 1] - 1) // P)
            for c in range(c0, c1 + 1):
                tile_chunks[t].append((pi, c, mcol, 0))
                mcol_map[pi][c, t] = mcol
                mcol += 1
    tile_chunks2 = [[] for _ in range(NT)]
    for t in range(NT):
        for gi in (0, 1):
            for j in range(8):
                g = t * 8 + j
                if maxcnt2[gi, g] == 0:
                    continue
                c0 = int(S2[gi, g] // P)
                c1 = int((S2[gi, g + 1] - 1) // P)
                for c in range(c0, c1 + 1):
                    tile_chunks2[t].append((gi, c, mcol, j * FO))
                    mcol_map2[gi][c, g] = mcol
                    mcol += 1
    M_total = mcol

    # per-core streams: gather idx tables + selection values
    idx16 = []
    selv = []
    for c in range(NCORES):
        iv = np.zeros((CL + CH + CA2 + CB2) * P, dtype=np.int64)
        sv = np.full((P, M_total), -1.0, dtype=np.float32)
        for pi, m in ((0, (core == c) & lo), (1, (core == c) & ~lo)):
            te = tile_e[m]
            order = np.argsort(te, kind="stable")
            te_s = te[order]
            rows_s = rows[m][order] - (0 if pi == 0 else LOWROWS)
            dl_s = dlane[m][order]
            starts = np.searchsorted(te_s, np.arange(NT))
            pos = S[pi][te_s] + (np.arange(te_s.size) - starts[te_s])
            base = 0 if pi == 0 else CL * P
            iv[base + pos] = rows_s
            ch = pos // P
            mc = mcol_map[pi][ch, te_s]
            assert (mc >= 0).all()
            sv[pos % P, mc] = dl_s.astype(np.float32)
        # L2 streams: per src group, sorted by (tile, sub)
        for gi, m in ((0, (core == c) & ~grpB), (1, (core == c) & grpB)):
            g_e = tile_e[m] * 8 + sub2[m]
            order = np.argsort(g_e, kind="stable")
            g_s = g_e[order]
            idx2_s = idx2[m][order]
            dl_s = dlane[m][order]
            starts = np.searchsorted(g_s, np.arange(NT * 8))
            pos = S2[gi][g_s] + (np.arange(g_s.size) - starts[g_s])
            base = (CL + CH + (0 if gi == 0 else CA2)) * P
            iv[base + pos] = idx2_s
            mc = mcol_map2[gi][pos // P, g_s]
            assert (mc >= 0).all()
            sv[pos % P, mc] = dl_s.astype(np.float32)
        idx16.append(_pack_idx16(iv))
        selv.append(sv)

    # xpack [128, XCOLS] bf16: partition (a*64+f), col j <-> xs[a*XCOLS+j, f]
    xs = np.zeros((NPAD, F), dtype=np.float32)
    xs[:N] = np.asarray(x, dtype=np.float32) * dinv[:, None]
    xpack = xs.reshape(2, XCOLS, F).transpose(0, 2, 1).reshape(P, XCOLS).astype(BF16)

    dinv_s = np.zeros((NCORES, P, NT), dtype=np.float32)
    for c in range(NCORES):
        ll = np.arange(min(SH, N - c * SH))
        dinv_s[c, ll % P, ll // P] = dinv[c * SH : c * SH + ll.size]

    b1z = not np.any(np.asarray(b1))
    b2z = not np.any(np.asarray(b2))
    common = {
        "xpack": xpack,
        "w1": np.concatenate([np.asarray(W1, np.float32)] * 2, axis=0).astype(BF16),
        "w2": np.asarray(W2, dtype=np.float32).astype(BF16),
    }
    if not b1z:
        common["b1r"] = np.broadcast_to(np.asarray(b1, np.float32), (P, F)).copy()
    if not b2z:
        common["b2r"] = np.broadcast_to(np.asarray(b2, np.float32), (P, FO)).copy()
    in_maps = []
    for c in range(NCORES):
        m = dict(common)
        m["dinvs"] = dinv_s[c]
        m["idx16"] = idx16[c]
        m["selv"] = selv[c]
        in_maps.append(m)

    static = {
        "CL": CL,
        "CH": CH,
        "CA2": CA2,
        "CB2": CB2,
        "M": M_total,
        "tile_chunks": tile_chunks,
        "tile_chunks2": tile_chunks2,
        "b1z": b1z,
        "b2z": b2z,
    }
    return in_maps, static


_WAIT_LIMIT = int(os.environ.get("GCN_WAIT_LIMIT", "1"))


def _legalize_waits(nc, limit=None):
    """Split >limit semaphore waits into standalone NOPs on the same engine.

    Walrus codegen rejects instructions whose sync_info carries more wait
    conditions than the ISA sync fields hold ("Too many sync wait commands").
    A chain of no-ops each carrying <=limit waits is semantically identical
    (waits are AND conditions and the engine queue is in-order).
    """
    if limit is None:
        limit = _WAIT_LIMIT
    import bass_rust as _br

    uid = 0
    for fn in nc.m.functions:
        for bb in fn.blocks:
            out = []
            changed = False
            for ins in bb.instructions:
                si = ins.sync_info
                if si is not None and len(si.on_wait) > limit:
                    waits = list(si.on_wait)
                    excess, keep = waits[:-limit], waits[-limit:]
                    for i in range(0, len(excess), limit):
                        nop = mybir.InstNoOp(name=f"waitsplit_{uid}", ins=[], outs=[])
                        uid += 1
                        nop.engine = ins.engine
                        nop.sync_info = _br.SyncInfo(
                            on_wait=excess[i : i + limit], on_update=[]
                        )
                        out.append(nop)
                    ins.sync_info = _br.SyncInfo(
                        on_wait=keep, on_update=list(si.on_update)
                    )
                    changed = True
                out.append(ins)
            if changed:
                bb.instructions = out
    return nc


def _build(st):
    dt = mybir.dt
    Alu = mybir.AluOpType
    Act = mybir.ActivationFunctionType
    CL, CH, CA2, CB2 = st["CL"], st["CH"], st["CA2"], st["CB2"]
    M = st["M"]
    tile_chunks = st["tile_chunks"]
    tile_chunks2 = st["tile_chunks2"]
    b1z, b2z = st["b1z"], st["b2z"]

    nc = bass.Bass(num_devices=NCORES)

    # ---- I/O ----
    xpack_e = nc.dram_tensor("xpack", [P, XCOLS], dt.bfloat16, kind="ExternalInput")
    w1_e = nc.dram_tensor("w1", [2 * F, F], dt.bfloat16, kind="ExternalInput")
    w2_e = nc.dram_tensor("w2", [F, FO], dt.bfloat16, kind="ExternalInput")
    if not b1z:
        b1_e = nc.dram_tensor("b1r", [P, F], dt.float32, kind="ExternalInput")
    if not b2z:
        b2_e = nc.dram_tensor("b2r", [P, FO], dt.float32, kind="ExternalInput")
    dinvs_e = nc.dram_tensor("dinvs", [P, NT], dt.float32, kind="ExternalInput")
    idx16_e = nc.dram_tensor(
        "idx16", [P, (CL + CH + CA2 + CB2) * 8], dt.int16, kind="ExternalInput"
    )
    selv_e = nc.dram_tensor("selv", [P, M], dt.float32, kind="ExternalInput")
    out_e = nc.dram_tensor("out", [P, NT * FO], dt.float32, kind="ExternalOutput")

    # ---- internal DRAM ----
    # h_dram row (p*8+c)*49+t = 256B-padded h' row of node c*6272+t*128+p.
    # cc_out is COMPACT [50176, 16]: 4 contiguous AllGather piece regions
    # (collectives demand contiguous in/out); layer 2 gathers 256B = 8 rows
    # per descriptor, the wanted 32B row picked by a per-chunk rhs sub-slice.
    h_dram = nc.dram_tensor("h_dram", [NPAD, EPR], dt.bfloat16)
    cc_ins = [
        nc.dram_tensor(
            f"cc_in{k}", [P, (PB[k + 1] - PB[k]) * FO], dt.bfloat16
        )
        for k in range(4)
    ]
    NRA = NCORES * P * PB[3]            # compact rows in cc_outA (pieces 0-2)
    NRB = NCORES * P * (NT - PB[3])     # compact rows in cc_outB (piece 3)
    cc_outA = nc.dram_tensor(
        "cc_outA", [NRA, FO], dt.bfloat16, addr_space="Shared"
    )
    cc_outB = nc.dram_tensor(
        "cc_outB", [NRB, FO], dt.bfloat16, addr_space="Shared"
    )

    with tile.TileContext(nc) as tc:
        with tc.tile_pool(name="const", bufs=1) as cp:
            w1 = cp.tile([2 * F, F], dt.bfloat16, tag="w1")
            nc.sync.dma_start(out=w1[:], in_=w1_e[:, :])
            w2 = cp.tile([F, FO], dt.bfloat16, tag="w2")
            nc.sync.dma_start(out=w2[:], in_=w2_e[:, :])
            if not b1z:
                b1r = cp.tile([P, F], dt.float32, tag="b1r")
                nc.sync.dma_start(out=b1r[:], in_=b1_e[:, :])
            if not b2z:
                b2r = cp.tile([P, FO], dt.float32, tag="b2r")
                nc.sync.dma_start(out=b2r[:], in_=b2_e[:, :])
            dinvs = cp.tile([P, NT], dt.float32, tag="dinvs")
            nc.sync.dma_start(out=dinvs[:], in_=dinvs_e[:, :])
            idx16 = cp.tile(
                [P, (CL + CH + CA2 + CB2) * 8], dt.int16, tag="idx16"
            )
            nc.sync.dma_start(out=idx16[:], in_=idx16_e[:, :])
            selv = cp.tile([P, M], dt.float32, tag="selv")
            nc.sync.dma_start(out=selv[:], in_=selv_e[:, :])
            iota = cp.tile([P, P], dt.bfloat16, tag="iota")
            nc.gpsimd.iota(
                iota[:],
                pattern=[[1, P]],
                base=0,
                channel_multiplier=0,
                allow_small_or_imprecise_dtypes=True,
            )
            ident = cp.tile([P, P], dt.bfloat16, tag="ident")
            make_identity(nc, ident[:])
            out1 = cp.tile([P, NT * F], dt.bfloat16, tag="out1")
            h2st = cp.tile([P, NT * FO], dt.bfloat16, tag="h2st")
            outst = cp.tile([P, NT * FO], dt.float32, tag="outst")

            tc.strict_bb_all_engine_barrier()
            # dma_gather lives in the Q7 "mlp" extended-instruction library.
            # bass's pseudo reload ships with an empty instr payload, which
            # walrus rejects ("ISA wrong length") — fill the 64B struct.
            _li = nc.gpsimd.load_library(library_config.mlp)
            _instr, _fx = bass_isa.isa_struct(
                nc.isa,
                nc.isa.Opcode.NEURON_ISA_TPB_OPCODE_PSEUDO_INST,
                {"pseudo_opcode": 2, "lib_index": library_config.mlp.index},
                struct_name="NEURON_ISA_TPB_PSEUDO_LIBRARY_RELOAD_INDEX_STRUCT",
            )
            _li.ins.instr = _instr

            # ------------- Phase A: h' = bf16((dinv*x) @ W1) -----------------
            XB = 14 * P  # node tiles per xpack block (14 tiles, 2 psum groups)
            with (
                tc.tile_pool(name="xa", bufs=3) as xpool,
                tc.tile_pool(name="ha", bufs=6) as hpool,
                tc.tile_pool(name="pha", bufs=6, space="PSUM") as phpool,
            ):
                gi = 0
                for blk in range(0, XCOLS, XB):
                    xb = xpool.tile([P, XB], dt.bfloat16, tag="xb")
                    nc.sync.dma_start(out=xb[:], in_=xpack_e[:, blk : blk + XB])
                    for a in (0, 1):
                        for g0 in range(0, XB // P, TPG):
                            ph = phpool.tile([P, TPG * F], dt.float32, tag="ph")
                            for i in range(TPG):
                                m = g0 + i
                                nc.tensor.matmul(
                                    out=ph[:, i * F : (i + 1) * F],
                                    lhsT=xb[
                                        a * F : (a + 1) * F, m * P : (m + 1) * P
                                    ],
                                    rhs=w1[a * F : (a + 1) * F, :],
                                    start=True,
                                    stop=True,
                                )
                            hst = hpool.tile([P, TPG * F], dt.bfloat16, tag="hst")
                            # psum->sbuf bf16 casts on DVE (idle in phase A);
                            # writes alternate between the Act and SP queues
                            # so no single SEQ paces the pipeline
                            nc.vector.tensor_copy(out=hst[:], in_=ph[:])
                            tg0 = a * XH + blk // P + g0
                            c_, t0 = tg0 // NT, tg0 % NT
                            dst = bass.AP(
                                h_dram,
                                (c_ * NT + t0) * EPR,
                                [[NCORES * NT * EPR, P], [EPR, TPG], [1, F]],
                            )
                            eng = nc.scalar if gi % 2 == 0 else nc.sync
                            eng.dma_start(out=dst, in_=hst[:])
                            gi += 1

            # ------------- shared aggregation machinery ----------------------
            nidx_regs = {}
            cc_delay = []  # [BassInstruction, remaining gather-blocks]

            def aggregation(passes, tile_entries, fw, psum_pool,
                            gpool, selpool, tailpool, pool_every=0):
                # passes: per gather pass (nchunks, src_ap, idx chunk base)
                gbufs = {}

                def blocks_for(nch):
                    # full G-chunk blocks, but a tail of small blocks so the
                    # end-of-pass pipeline drain is short
                    bl, c0 = [], 0
                    while c0 < nch:
                        w = G if nch - c0 > 40 else min(8, nch - c0)
                        bl.append((c0, w))
                        c0 += w
                    return bl

                blocks = [blocks_for(p[0]) for p in passes]
                blk_of = []
                for pi, p in enumerate(passes):
                    m_ = np.zeros(p[0], dtype=np.int64)
                    for bi, (c0, w) in enumerate(blocks[pi]):
                        m_[c0 : c0 + w] = bi
                    blk_of.append(m_)

                def issue(pi, b):
                    nch, src, base = passes[pi]
                    c0, w = blocks[pi][b]
                    gb = gpool.tile([P, w * EPR], dt.bfloat16, tag=f"gb{w}")
                    # one shared Pool register per distinct idx count (a fresh
                    # to_reg per gather exhausts the register file)
                    if w * P not in nidx_regs:
                        nidx_regs[w * P] = nc.gpsimd.to_reg(w * P)
                    gin = nc.gpsimd.dma_gather(
                        out_ap=gb[:, : w * EPR].rearrange("p (s e) -> p s e", e=EPR),
                        in_ap=src,
                        idxs_ap=idx16[:, (base + c0) * 8 : (base + c0 + w) * 8],
                        num_idxs=w * P,
                        num_idxs_reg=nidx_regs[w * P],
                        elem_size=EPR,
                        single_packet=False,
                    )
                    gbufs[(pi, b)] = gb
                    # attach any pending collective so it lands in Pool's
                    # stream only after enough gather desc-gen has been
                    # issued (its cc_in wait is then already satisfied)
                    for ent in cc_delay:
                        ent[1] -= 1
                        if ent[1] == 0:
                            add_dep_helper(
                                ent[0].ins, gin.ins, sync=False,
                                reason="delay collective past gathers",
                            )
                    cc_delay[:] = [e for e in cc_delay if e[1] > 0]

                nsel = [0]

                def build_sel(pool, mc, allow_pool=False):
                    # offload a fraction of builds to the gpsimd engine when
                    # the layer is DVE-bound
                    sel = pool.tile([P, P], dt.bfloat16, tag="sel")
                    nsel[0] += 1
                    eng = (
                        nc.gpsimd
                        if allow_pool and pool_every and nsel[0] % pool_every == 0
                        else nc.vector
                    )
                    eng.tensor_scalar(
                        out=sel[:],
                        in0=iota[:],
                        scalar1=selv[:, mc : mc + 1],
                        scalar2=None,
                        op0=Alu.is_equal,
                    )
                    return sel

                # prebuild the last tiles' selection matrices up front: at
                # drain time DVE's in-order queue is otherwise still full of
                # them (blocked behind per-tile epilogue reduces)
                tail_sels = {}
                for t in range(NT - 6, NT):
                    for ent in sorted(tile_entries[t], key=lambda e: -e[0]):
                        if len(tail_sels) < 120:
                            tail_sels[ent[2]] = build_sel(tailpool, ent[2])

                def sweep(entries_of, epilogue, tag):
                    for t in range(NT):
                        ents = entries_of(t)
                        if not ents:
                            epilogue(t, None)
                            continue
                        pt = psum_pool.tile([P, fw], dt.float32, tag=tag)
                        nmm = len(ents)
                        for k, (pi, c, mc, roff) in enumerate(ents):
                            b = int(blk_of[pi][c])
                            if (pi, b) not in gbufs:
                                issue(pi, b)
                            c0b = blocks[pi][b][0]
                            rhs = gbufs[(pi, b)][
                                :,
                                (c - c0b) * EPR + roff :
                                (c - c0b) * EPR + roff + fw,
                            ]
                            sel = tail_sels.get(mc)
                            if sel is None:
                                sel = build_sel(selpool, mc, allow_pool=True)
                            nc.tensor.matmul(
                                out=pt[:],
                                lhsT=sel[:],
                                rhs=rhs,
                                start=(k == 0),
                                stop=(k == nmm - 1),
                            )
                        epilogue(t, pt)

                return sweep

            # ------------- Layer 1 aggregation + transform + collective ------
            ndone = [0]
            with (
                tc.tile_pool(name="gb1", bufs=5) as gpool1,
                tc.tile_pool(name="sel1", bufs=96) as selpool1,
                tc.tile_pool(name="selt1", bufs=128) as tailpool1,
                tc.tile_pool(name="pagg1", bufs=6, space="PSUM") as pp1,
                tc.tile_pool(name="ptr", bufs=1, space="PSUM") as ptrpool,
                tc.tile_pool(name="ph2", bufs=1, space="PSUM") as ph2pool,
                tc.tile_pool(name="o1t", bufs=2) as o1tpool,
                tc.tile_pool(name="ep1", bufs=2) as ep1pool,
            ):

                def fire_piece(k):
                    # AllGather piece (Pool engine — the only one walrus
                    # allows; in/out must be contiguous).  Piece k's output
                    # is a contiguous region of the compact cc_out.  An
                    # artificial dep pushes the instruction ~7 gather blocks
                    # past its milestone in Pool's stream so its cc_in wait
                    # doesn't stall desc-gen.
                    t0, t1 = PB[k], PB[k + 1]
                    w = (t1 - t0) * FO
                    cct = cc_outA if k < 3 else cc_outB
                    off = 0 if k == 3 else NCORES * P * FO * t0
                    nc.sync.dma_start(
                        out=cc_ins[k][:, :], in_=h2st[:, t0 * FO : t1 * FO]
                    )
                    cc = nc.gpsimd.collective_compute(
                        "AllGather",
                        mybir.AluOpType.bypass,
                        replica_groups=[list(range(NCORES))],
                        ins=[cc_ins[k][:, :]],
                        outs=[
                            bass.AP(
                                cct,
                                off,
                                [[P * w, NCORES], [w, P], [1, w]],
                            )
                        ],
                    )
                    if k < 3:
                        cc_delay.append([cc, 7])

                def on_stop1(t, pt):
                    # epilogue: out1 = relu(dinv_d * sum (+ b1))
                    if b1z:
                        nc.scalar.activation(
                            out=out1[:, t * F : (t + 1) * F],
                            in_=pt[:],
                            func=Act.Relu,
                            scale=dinvs[:, t : t + 1],
                        )
                    else:
                        tmp = ep1pool.tile([P, F], dt.float32, tag="tmp")
                        nc.vector.tensor_scalar(
                            out=tmp[:],
                            in0=pt[:],
                            scalar1=dinvs[:, t : t + 1],
                            scalar2=None,
                            op0=Alu.mult,
                        )
                        nc.vector.tensor_tensor(
                            out=tmp[:], in0=tmp[:], in1=b1r[:], op=Alu.add
                        )
                        nc.scalar.activation(
                            out=out1[:, t * F : (t + 1) * F],
                            in_=tmp[:],
                            func=Act.Relu,
                        )
                    # layer-2 transform for this tile: h2' = dinv_d*(out1@W2)
                    ptr_ = ptrpool.tile([P, P], dt.bfloat16, tag="ptr")
                    nc.tensor.transpose(
                        out=ptr_[:F, :],
                        in_=out1[:, t * F : (t + 1) * F],
                        identity=ident[:],
                    )
                    o1T = o1tpool.tile([F, P], dt.bfloat16, tag="o1T")
                    nc.vector.tensor_copy(out=o1T[:], in_=ptr_[:F, :])
                    ph2 = ph2pool.tile([P, FO], dt.float32, tag="ph2")
                    nc.tensor.matmul(
                        out=ph2[:], lhsT=o1T[:], rhs=w2[:, :], start=True, stop=True
                    )
                    nc.scalar.activation(
                        out=h2st[:, t * FO : (t + 1) * FO],
                        in_=ph2[:],
                        func=Act.Copy,
                        scale=dinvs[:, t : t + 1],
                    )
                    ndone[0] += 1
                    for k in range(4):
                        if ndone[0] == PB[k + 1]:
                            fire_piece(k)

                passes1 = [
                    (CL, h_dram.ap(), 0),
                    (
                        CH,
                        bass.AP(
                            h_dram,
                            LOWROWS * EPR,
                            [[EPR, NPAD - LOWROWS], [1, EPR]],
                        ),
                        CL,
                    ),
                ]
                sweep1 = aggregation(passes1, tile_chunks, F, pp1,
                                     gpool1, selpool1, tailpool1)
                sweep1(lambda t: tile_chunks[t], on_stop1, "agg")

            # ------------- Layer 2 aggregation + log_softmax -----------------
            with (
                tc.tile_pool(name="gb2", bufs=5) as gpool2,
                tc.tile_pool(name="sel2", bufs=240) as selpool2,
                tc.tile_pool(name="selt2", bufs=128) as tailpool2,
                tc.tile_pool(name="pagg2", bufs=6, space="PSUM") as pp2,
                tc.tile_pool(name="ep2", bufs=8) as ep2pool,
            ):

                accA = cp.tile([P, NT * FO], dt.float32, tag="accA")
                accB = cp.tile([P, NT * FO], dt.float32, tag="accB")

                def flush_out(t):
                    # flush finished outst columns piecewise so the final
                    # write isn't one big end-serialized DMA
                    for q0, q1 in ((0, 13), (13, 26), (26, 38), (38, 49)):
                        if t == q1 - 1:
                            nc.sync.dma_start(
                                out=out_e[:, q0 * FO : q1 * FO],
                                in_=outst[:, q0 * FO : q1 * FO],
                            )

                def spill_to(acc):
                    # spill partial sums (pre-scaled by dinv_d) on Act, so
                    # DVE's in-order queue carries ONLY sel builds during the
                    # gather sweeps (no head-of-line epilogue waits)
                    def spill(t, pt):
                        if pt is None:
                            nc.gpsimd.memset(acc[:, t * FO : (t + 1) * FO], 0.0)
                            return
                        nc.scalar.activation(
                            out=acc[:, t * FO : (t + 1) * FO],
                            in_=pt[:],
                            func=Act.Copy,
                            scale=dinvs[:, t : t + 1],
                        )
                    return spill

                def epi_final(t):
                    # log_softmax from the two spilled partials
                    tmp = ep2pool.tile([P, FO], dt.float32, tag="tmp2")
                    nc.vector.tensor_tensor(
                        out=tmp[:],
                        in0=accA[:, t * FO : (t + 1) * FO],
                        in1=accB[:, t * FO : (t + 1) * FO],
                        op=Alu.add,
                    )
                    if not b2z:
                        nc.vector.tensor_tensor(
                            out=tmp[:], in0=tmp[:], in1=b2r[:], op=Alu.add
                        )
                    mx = ep2pool.tile([P, 1], dt.float32, tag="mx")
                    nc.vector.reduce_max(
                        out=mx[:], in_=tmp[:], axis=mybir.AxisListType.X, negate=True
                    )
                    ex = ep2pool.tile([P, FO], dt.float32, tag="ex")
                    nc.scalar.activation(
                        out=ex[:], in_=tmp[:], func=Act.Exp, bias=mx[:, 0:1]
                    )
                    sm = ep2pool.tile([P, 1], dt.float32, tag="sm")
                    nc.vector.reduce_sum(
                        out=sm[:], in_=ex[:], axis=mybir.AxisListType.X
                    )
                    lg = ep2pool.tile([P, 1], dt.float32, tag="lg")
                    nc.scalar.activation(out=lg[:], in_=sm[:], func=Act.Ln)
                    nc.vector.tensor_scalar(
                        out=outst[:, t * FO : (t + 1) * FO],
                        in0=tmp[:],
                        scalar1=mx[:, 0:1],
                        scalar2=lg[:, 0:1],
                        op0=Alu.add,
                        op1=Alu.subtract,
                    )
                    flush_out(t)

                passes2 = [
                    (
                        CA2,
                        bass.AP(cc_outA, 0, [[EPR, NRA * FO // EPR], [1, EPR]]),
                        CL + CH,
                    ),
                    (
                        CB2,
                        bass.AP(cc_outB, 0, [[EPR, NRB * FO // EPR], [1, EPR]]),
                        CL + CH + CA2,
                    ),
                ]
                sweep2 = aggregation(passes2, tile_chunks2, FO, pp2,
                                     gpool2, selpool2, tailpool2,
                                     pool_every=5)
                # sweep A first (src pieces 0-2, available before the last
                # collective lands), then sweep B, then the epilogues
                sweep2(
                    lambda t: [e for e in tile_chunks2[t] if e[0] == 0],
                    spill_to(accA), "agg",
                )
                sweep2(
                    lambda t: [e for e in tile_chunks2[t] if e[0] == 1],
                    spill_to(accB), "agg",
                )
                for t in range(NT):
                    epi_final(t)

    _legalize_waits(nc)
    return nc


def kernel(x, edge_index, W1, b1, W2, b2, _trace=False, _trace_kwargs=None):
    in_maps, st = _preprocess(x, edge_index, W1, b1, W2, b2)
    key = (
        st["CL"],
        st["CH"],
        st["CA2"],
        st["CB2"],
        st["M"],
        st["b1z"],
        st["b2z"],
        tuple(tuple(tc_) for tc_ in map(tuple, st["tile_chunks"])),
        tuple(tuple(tc_) for tc_ in map(tuple, st["tile_chunks2"])),
    )
    if key not in _CACHE:
        _CACHE[key] = _build(st)
    nc = _CACHE[key]

    res = run_bass_kernel_spmd(
        nc,
        in_maps,
        core_ids=list(range(NCORES)),
        trace=_trace,
        **(_trace_kwargs or {}),
    )
    out = np.empty((N, FO), dtype=np.float32)
    for c in range(NCORES):
        o = np.asarray(res.results[c]["out"], dtype=np.float32)
        o = o.reshape(P, NT, FO).transpose(1, 0, 2).reshape(NT * P, FO)
        k = min(SH, N - c * SH)
        out[c * SH : c * SH + k] = o[:k]
    kernel._last_result = res
    return out
